# revision 16
# baseline (speedup 1.0000x reference)
"""nn_MultiHeadAttention_59253368815813 on 8 TRN2 NeuronCores.

The reference module is bug-faithful to its original nn.Module in two ways
that together collapse the computation:

  1. ``o = jnp.einsum('bhtl,bthd->bhtd', A, v)`` indexes ``v`` by the QUERY
     position ``t``, not the key position ``l``; the softmax weights sum to
     1, so ``o[b,h,t,d] == v[b,t,h,d]``. Q, K, mask, softmax are dead.
  2. ``o.reshape(b, T, d)`` with no transpose scrambles (head, token): row
     tj = 128*h + s is the concat over m of v[b, 16*s+m, h, :].

So the computation is  out = scramble(x @ Wv) @ Wo.T  and output rows
depend on one head only.

Sharding: 2 batches x 4 head-groups. Core c = (b=c//4, g=c%4) owns batch b,
heads {4g..4g+3} (Wv cols [256g,256g+256), out rows [512g,512g+512) of b).
Each core loads only its batch's x, so the kernel is PE-bound, not
DMA-bound.

Precision/speed: fp8e4m3 DoubleRow matmuls (0.5 cycles/row, 2x bf16) with
hi+lo error compensation: A@B ~= Ah@Bh + Al@Bh + Ah@Bl where Xh=fp8(X),
Xl=fp8(X-Xh). That is 12 DoubleRow matmuls per 8-chunk contraction vs 8
bf16 matmuls - 25% less PE time at ~bf16 accuracy (~4e-3 rel, gate 2e-2).
Wv/Wo are pre-scaled by 16 on the host so all fp8 operands sit in normal
range; psum holds 16v / 256out and the output flush scales by 1/256.

Per core: x^T (tokens permuted to u = 128m + s) streams in 4 blocks of 512
tokens (hi+lo fp8); v-proj DoubleRows chase the stream; evacuation copies
write 16v to bf16 vt (DVE/Act); gpsimd+DVE/Act split vt into fp8 hi/lo;
out-proj cols [0,512) accumulates during the stream (block k = chunk pair
(2k,2k+1)), cols [512,1024) run after, overlapping the output DMAs.
"""

import sys
import types

import numpy as np

_TRN_REPO = "/opt/trn_rl_repo"
if _TRN_REPO not in sys.path:
    sys.path.insert(0, _TRN_REPO)


def _install_ntff_shim():
    """antenv.axon_hooks is absent in this container; provide it so
    BASS_TRACE=1 profiling works. No-op if the real module exists."""
    try:
        import antenv  # noqa: F401
    except ImportError:
        return
    if "antenv.axon_hooks" in sys.modules:
        return
    try:
        import antenv.axon_hooks  # noqa: F401
        return
    except ImportError:
        pass
    m = types.ModuleType("antenv.axon_hooks")
    m._hook = None
    m.set_axon_ntff_profile_hook = lambda h: setattr(m, "_hook", h)
    m.get_axon_ntff_profile_hook = lambda: m._hook
    sys.modules["antenv.axon_hooks"] = m
    try:
        from trn_agent_boot.trn_boot import _ntff_profile_via_ctypes

        hook = _ntff_profile_via_ctypes("/opt/axon/libaxon_pjrt.so")
        if hook is not None:
            m.set_axon_ntff_profile_hook(hook)
    except Exception:
        pass


_install_ntff_shim()

import concourse.mybir as mybir  # noqa: E402
import concourse.tile as tile  # noqa: E402
from concourse import bacc  # noqa: E402
from concourse.bass_utils import run_bass_kernel_spmd  # noqa: E402

F32 = mybir.dt.float32
BF16 = mybir.dt.bfloat16
F8 = mybir.dt.float8e4
BF = mybir.dt.np(BF16)
F8NP = mybir.dt.np(F8)
DR = mybir.MatmulPerfMode.DoubleRow
MULT = mybir.AluOpType.mult
SUB = mybir.AluOpType.subtract

B = 2
T = 2048
D = 1024
NCORES = 8
NB = 4       # 512-token (u) blocks per batch
UB = 512
NC8 = 8      # contraction chunks (d = 8*128)
NH = 4       # local heads per core

_CACHED = None
LAST_RESULTS = None


def _build_module():
    nc = bacc.Bacc("TRN2", target_bir_lowering=False, debug=False,
                   num_devices=NCORES)

    xh_d = nc.dram_tensor("xh", [NB, 128, NC8, UB], F8,
                          kind="ExternalInput").ap()
    xl_d = nc.dram_tensor("xl", [NB, 128, NC8, UB], F8,
                          kind="ExternalInput").ap()
    wvh_d = nc.dram_tensor("wvh", [128, NC8, 256], F8,
                           kind="ExternalInput").ap()
    wvl_d = nc.dram_tensor("wvl", [128, NC8, 256], F8,
                           kind="ExternalInput").ap()
    woh_d = nc.dram_tensor("woh", [128, 8, D], F8, kind="ExternalInput").ap()
    wol_d = nc.dram_tensor("wol", [128, 8, D], F8, kind="ExternalInput").ap()
    out_d = nc.dram_tensor("out", [NH, 128, D], BF16,
                           kind="ExternalOutput").ap()

    with tile.TileContext(nc) as tc:
        _emit(nc, tc, xh_d, xl_d, wvh_d, wvl_d, woh_d, wol_d, out_d)
    nc.compile()
    return nc


def _emit(nc, tc, xh_d, xl_d, wvh_d, wvl_d, woh_d, wol_d, out_d):
    from contextlib import ExitStack

    ctx = ExitStack()
    with ctx:
        wpool = ctx.enter_context(tc.tile_pool(name="w", bufs=1))
        xtp = ctx.enter_context(tc.tile_pool(name="xt", bufs=NB))
        vtp = ctx.enter_context(tc.tile_pool(name="vt", bufs=1))
        outp = ctx.enter_context(tc.tile_pool(name="outsb", bufs=4))
        ps_v = ctx.enter_context(tc.tile_pool(name="ps_v", bufs=4, space="PSUM"))
        ps_o = ctx.enter_context(tc.tile_pool(name="ps_o", bufs=4, space="PSUM"))

        # wv split so the first v-matmuls only wait on the small first piece
        wvh1 = wpool.tile([128, 2, 256], F8, tag="wvh1")
        wvh2 = wpool.tile([128, 6, 256], F8, tag="wvh2")
        wvl1 = wpool.tile([128, 2, 256], F8, tag="wvl1")
        wvl2 = wpool.tile([128, 6, 256], F8, tag="wvl2")
        woh = wpool.tile([128, 8, D], F8, tag="woh")
        wol = wpool.tile([128, 8, D], F8, tag="wol")

        def wv_pair(hi, cp, hp):
            """lhsT [128, 2, 128] = Wv16 chunk pair (2cp, 2cp+1), head-pair hp."""
            t1, t2 = (wvh1, wvh2) if hi else (wvl1, wvl2)
            if cp == 0:
                return t1[:, :, 128 * hp:128 * hp + 128]
            return t2[:, 2 * cp - 2:2 * cp, 128 * hp:128 * hp + 128]

        xh0a = xtp.tile([128, 4, UB], F8, tag="xh0a")
        xh0b = xtp.tile([128, 4, UB], F8, tag="xh0b")
        xl0a = xtp.tile([128, 4, UB], F8, tag="xl0a")
        xl0b = xtp.tile([128, 4, UB], F8, tag="xl0b")
        xh = [None] + [xtp.tile([128, NC8, UB], F8, tag="xh", name=f"xh{k}")
                       for k in range(1, NB)]
        xl = [None] + [xtp.tile([128, NC8, UB], F8, tag="xl", name=f"xl{k}")
                       for k in range(1, NB)]

        def x_pair(hi, k, cp):
            """rhs [128, 2, 512] = x chunk pair (2cp, 2cp+1) of block k."""
            if k == 0:
                a, bb = (xh0a, xh0b) if hi else (xl0a, xl0b)
                t = a if cp < 2 else bb
                i = 2 * (cp % 2)
                return t[:, i:i + 2, :]
            t = xh[k] if hi else xl[k]
            return t[:, 2 * cp:2 * cp + 2, :]

        # x stream on the sync/SP queue, weights on the Act queue (parallel)
        nc.sync.dma_start(xh0a[:], xh_d[0, :, 0:4, :])
        nc.sync.dma_start(xl0a[:], xl_d[0, :, 0:4, :])
        nc.sync.dma_start(xh0b[:], xh_d[0, :, 4:8, :])
        nc.sync.dma_start(xl0b[:], xl_d[0, :, 4:8, :])
        for k in range(1, NB):
            nc.sync.dma_start(xh[k][:], xh_d[k])
            nc.sync.dma_start(xl[k][:], xl_d[k])
        nc.scalar.dma_start(wvh1[:], wvh_d[:, 0:2, :])
        nc.scalar.dma_start(wvl1[:], wvl_d[:, 0:2, :])
        nc.scalar.dma_start(wvh2[:], wvh_d[:, 2:8, :])
        nc.scalar.dma_start(wvl2[:], wvl_d[:, 2:8, :])
        for q in range(4):
            nc.scalar.dma_start(woh[:, 2 * q:2 * q + 2, :],
                                woh_d[:, 2 * q:2 * q + 2, :])
            nc.scalar.dma_start(wol[:, 2 * q:2 * q + 2, :],
                                wol_d[:, 2 * q:2 * q + 2, :])

        # vt[h][64*(m%2)+di, 128*(m//2)+s] = 16*v[t=16s+m, 256g+64h+di], bf16
        vt = [vtp.tile([128, D], BF16, tag=f"vt{h}", name=f"vt{h}")
              for h in range(NH)]
        # fp8 hi/lo of vt, chunk-pair layout for DoubleRow lhsT
        vt8 = [vtp.tile([128, NC8, 2, 128], F8, tag=f"vt8{h}",
                        name=f"vt8{h}") for h in range(NH)]

        psA = [ps_o.tile([128, 512], F32, tag="po", name=f"psA{h}")
               for h in range(NH)]

        copy_engines = [nc.vector, nc.scalar]
        ce = [0]

        def ecopy(dst, src, scale=None):
            eng = copy_engines[ce[0] % 2]
            ce[0] += 1
            if eng is nc.scalar:
                if scale is None:
                    eng.copy(dst, src)
                else:
                    eng.mul(dst, src, scale)
            else:
                if scale is None:
                    eng.tensor_copy(dst, src)
                else:
                    eng.tensor_scalar_mul(dst, src, scale)

        VAR = [(1, 1), (1, 0), (0, 1)]  # (hi_w, hi_x): hh, hl, lh

        def vblock(k):
            psv = [ps_v.tile([128, UB], F32, tag="pv", name=f"pv{k}_{hp}")
                   for hp in range(2)]
            for cp in range(4):
                for vi, (hi_w, hi_x) in enumerate(VAR):
                    for hp in range(2):
                        nc.tensor.matmul(
                            psv[hp][:], wv_pair(hi_w, cp, hp),
                            x_pair(hi_x, k, cp),
                            start=(cp == 0 and vi == 0),
                            stop=(cp == 3 and vi == 2), perf_mode=DR)
            return psv

        def evac(k, psv):
            # block k holds m in {4k..4k+3}; j = m%2; chunk m2 = 2k + (m-4k)//2
            for hp in range(2):
                for hh in range(2):
                    for j in range(2):
                        for i in range(2):  # m - 4k = j + 2i
                            ecopy(vt[2 * hp + hh][
                                      64 * j:64 * j + 64,
                                      256 * k + 128 * i:256 * k + 128 * i + 128],
                                  psv[hp][64 * hh:64 * hh + 64,
                                          128 * (j + 2 * i):
                                          128 * (j + 2 * i) + 128])

        def split(k):
            # fp8 hi (DVE/Act) + lo = vt - hi (gpsimd) for chunks 2k, 2k+1
            for h in range(NH):
                for i in range(2):
                    c = 2 * k + i
                    src = vt[h][:, 128 * c:128 * c + 128]
                    ecopy(vt8[h][:, c, 0, :], src)
                    nc.gpsimd.tensor_sub(
                        vt8[h][:, c, 1, :], src, vt8[h][:, c, 0, :])

        def outA(k):
            for h in range(NH):
                lh = vt8[h][:, 2 * k:2 * k + 2, 0, :]
                ll = vt8[h][:, 2 * k:2 * k + 2, 1, :]
                rh = woh[:, 2 * k:2 * k + 2, 0:512]
                rl = wol[:, 2 * k:2 * k + 2, 0:512]
                nc.tensor.matmul(psA[h][:], lh, rh, start=(k == 0),
                                 stop=False, perf_mode=DR)
                nc.tensor.matmul(psA[h][:], ll, rh, start=False, stop=False,
                                 perf_mode=DR)
                nc.tensor.matmul(psA[h][:], lh, rl, start=False,
                                 stop=(k == NB - 1), perf_mode=DR)

        def flushA(h):
            ob = outp.tile([128, 512], BF16, tag="ob", name=f"obA{h}")
            ecopy(ob[:], psA[h][:], scale=1.0 / 256)
            nc.scalar.dma_start(out_d[h, :, 0:512], ob[:])

        # stream
        psv = vblock(0)
        evac(0, psv)
        split(0)
        for k in range(1, NB):
            psv = vblock(k)
            evac(k, psv)
            split(k)
            outA(k - 1)
        outA(NB - 1)

        for h in range(NH):
            flushA(h)

        # second pass: out-proj columns [512,1024)
        for h in range(NH):
            psB = ps_o.tile([128, 512], F32, tag="po", name=f"psB{h}")
            for q in range(4):
                lh = vt8[h][:, 2 * q:2 * q + 2, 0, :]
                ll = vt8[h][:, 2 * q:2 * q + 2, 1, :]
                rh = woh[:, 2 * q:2 * q + 2, 512:1024]
                rl = wol[:, 2 * q:2 * q + 2, 512:1024]
                nc.tensor.matmul(psB[:], lh, rh, start=(q == 0), stop=False,
                                 perf_mode=DR)
                nc.tensor.matmul(psB[:], ll, rh, start=False, stop=False,
                                 perf_mode=DR)
                nc.tensor.matmul(psB[:], lh, rl, start=False,
                                 stop=(q == 3), perf_mode=DR)
            ob = outp.tile([128, 512], BF16, tag="ob", name=f"obB{h}")
            ecopy(ob[:], psB[:], scale=1.0 / 256)
            nc.scalar.dma_start(out_d[h, :, 512:1024], ob[:])


def _get_module():
    global _CACHED
    if _CACHED is None:
        _CACHED = _build_module()
    return _CACHED


def _hilo(a):
    """fp8e4m3 hi + lo split of a float32 array."""
    hi = a.astype(F8NP)
    lo = (a - hi.astype(np.float32)).astype(F8NP)
    return hi, lo


def kernel(x, mask, Wq, Wk, Wv, Wo):
    global LAST_RESULTS
    x = np.asarray(x, dtype=np.float32)
    Wv = np.asarray(Wv, dtype=np.float32)
    Wo = np.asarray(Wo, dtype=np.float32)

    b, t, d = x.shape
    assert (b, t, d) == (B, T, D), (b, t, d)

    # x^T with tokens permuted to u = 128m + s (original t = 16s + m),
    # laid out [k, p, c8, u] to match the SBUF tiles exactly
    xhs, xls = [], []
    for bb in range(B):
        xT = x[bb].T
        xTp = xT.reshape(D, 128, 16).transpose(0, 2, 1).reshape(D, T)
        xt = np.ascontiguousarray(
            xTp.reshape(NC8, 128, NB, UB).transpose(2, 1, 0, 3))
        hi, lo = _hilo(xt)
        xhs.append(hi)
        xls.append(lo)

    # wv[p, c8, col] = 16*Wv[128*c8 + p, col]; per-core slice of 256 cols
    wvp = 16.0 * Wv.reshape(NC8, 128, D).transpose(1, 0, 2)
    # wo[p, m2, n] = 16*Wo.T[128*m2 + p, n]
    woh, wol = _hilo(np.ascontiguousarray(
        16.0 * Wo.T.reshape(8, 128, D).transpose(1, 0, 2)))

    in_maps = []
    for c in range(NCORES):
        bb, g = c // 4, c % 4
        wvh, wvl = _hilo(np.ascontiguousarray(
            wvp[:, :, 256 * g:256 * g + 256]))
        in_maps.append({
            "xh": xhs[bb], "xl": xls[bb],
            "wvh": wvh, "wvl": wvl,
            "woh": woh, "wol": wol,
        })

    nc = _get_module()
    res = run_bass_kernel_spmd(nc, in_maps, list(range(NCORES)))
    LAST_RESULTS = res

    out = np.empty((B, T, D), np.float32)
    for c in range(NCORES):
        bb, g = c // 4, c % 4
        out[bb, 512 * g:512 * g + 512, :] = \
            np.asarray(res.results[c]["out"]).astype(np.float32).reshape(512, D)
    return out


# revision 18
# speedup vs baseline: 1.2591x; 1.2591x over previous
"""nn_MultiHeadAttention_59253368815813 on 8 TRN2 NeuronCores.

The reference module is bug-faithful to its original nn.Module in two ways
that together collapse the computation:

  1. ``o = jnp.einsum('bhtl,bthd->bhtd', A, v)`` indexes ``v`` by the QUERY
     position ``t``, not the key position ``l``. ``l`` therefore only sums
     over the softmax weights, which sum to exactly 1 per row:
     ``o[b,h,t,d] == v[b,t,h,d]``. Q, K, the mask and the softmax never
     influence the output.
  2. ``o.reshape(b, T, d)`` with no transpose scrambles (head, token) so the
     reshaped activation row tj = 128*h + s is the concatenation over
     m=0..15 of v[b, 16*s+m, h, :].

So the exact computation is  out = scramble(x @ Wv) @ Wo.T,  and the
scramble makes output rows depend on one head only.

Sharding: 2 batches x 4 head-groups. Core c = (b=c//4, g=c%4) owns batch b
and heads {4g..4g+3} = Wv columns [256g, 256g+256) and output rows
[512g, 512g+512) of batch b. Each core loads only its batch's x (4.2MB in
bf16) instead of all of x, which is what made the previous version
DMA-bound (23.3MB/core at a shared ~360GB/s).

Per core, all in bf16 (PE runs bf16 at 1 cycle/row like f32r, but DMA
halves; quantization error ~2e-3 << the 2e-2 gate):
  stream x^T (tokens permuted to u = 128m + s, t = 16s + m) in 8 blocks of
  256 tokens; v-proj psum [128,256] per head-pair chases the stream; the
  reshape scramble happens in the psum->SBUF evacuation copies (spread over
  DVE/Pool/Act engines); output-projection columns [0,512) accumulate
  interleaved with the stream (chunk k uses only v tokens of block k);
  columns [512,1024) run as a second pass after the stream, overlapping the
  output DMAs.
"""

import sys
import types

import numpy as np

_TRN_REPO = "/opt/trn_rl_repo"
if _TRN_REPO not in sys.path:
    sys.path.insert(0, _TRN_REPO)


def _install_ntff_shim():
    """antenv.axon_hooks is absent in this container; provide it so
    BASS_TRACE=1 profiling works. No-op if the real module exists."""
    try:
        import antenv  # noqa: F401
    except ImportError:
        return
    if "antenv.axon_hooks" in sys.modules:
        return
    try:
        import antenv.axon_hooks  # noqa: F401
        return
    except ImportError:
        pass
    m = types.ModuleType("antenv.axon_hooks")
    m._hook = None
    m.set_axon_ntff_profile_hook = lambda h: setattr(m, "_hook", h)
    m.get_axon_ntff_profile_hook = lambda: m._hook
    sys.modules["antenv.axon_hooks"] = m
    try:
        from trn_agent_boot.trn_boot import _ntff_profile_via_ctypes

        hook = _ntff_profile_via_ctypes("/opt/axon/libaxon_pjrt.so")
        if hook is not None:
            m.set_axon_ntff_profile_hook(hook)
    except Exception:
        pass


_install_ntff_shim()

import ml_dtypes  # noqa: E402

import concourse.mybir as mybir  # noqa: E402
import concourse.tile as tile  # noqa: E402
from concourse import bacc  # noqa: E402
from concourse.bass_utils import run_bass_kernel_spmd  # noqa: E402

F32 = mybir.dt.float32
BF16 = mybir.dt.bfloat16
BF = ml_dtypes.bfloat16

B = 2
T = 2048
D = 1024
NCORES = 8
NB = 8       # 256-token (u) blocks per batch
UB = 256     # tokens per block
NC8 = 8      # contraction chunks (d = 8*128)
NH = 4       # local heads per core

_CACHED = None
LAST_RESULTS = None


def _build_module():
    nc = bacc.Bacc("TRN2", target_bir_lowering=False, debug=False,
                   num_devices=NCORES)

    xt_d = nc.dram_tensor("xt", [NB, 128, NC8, UB], BF16,
                          kind="ExternalInput").ap()
    wv_d = nc.dram_tensor("wv", [128, NC8, 256], BF16,
                          kind="ExternalInput").ap()
    wo_d = nc.dram_tensor("wo", [128, 8, D], BF16, kind="ExternalInput").ap()
    out_d = nc.dram_tensor("out", [NH, 128, D], BF16,
                           kind="ExternalOutput").ap()

    with tile.TileContext(nc) as tc:
        _emit(nc, tc, xt_d, wv_d, wo_d, out_d)
    nc.compile()
    return nc


def _emit(nc, tc, xt_d, wv_d, wo_d, out_d):
    from contextlib import ExitStack

    ctx = ExitStack()
    with ctx:
        wpool = ctx.enter_context(tc.tile_pool(name="w", bufs=1))
        xtp = ctx.enter_context(tc.tile_pool(name="xt", bufs=NB))
        vtp = ctx.enter_context(tc.tile_pool(name="vt", bufs=1))
        outp = ctx.enter_context(tc.tile_pool(name="outsb", bufs=4))
        ps_v = ctx.enter_context(tc.tile_pool(name="ps_v", bufs=4, space="PSUM"))
        ps_o = ctx.enter_context(tc.tile_pool(name="ps_o", bufs=4, space="PSUM"))

        # PE p-state warmup: the tensor engine clocks up only after ~3us of
        # continuous work, and the first real matmul cannot start until
        # ~10us (runtime preamble + first DMAs). Run throwaway matmuls on a
        # zeroed tile during that window so real matmuls run at full clock.
        warm_sb = wpool.tile([128, 256], BF16, tag="warm")
        nc.vector.memset(warm_sb[:], 0.0)
        warm_ps = ps_v.tile([128, UB], F32, tag="pv", name="warm_ps")
        for _ in range(50):
            nc.tensor.matmul(warm_ps[:], warm_sb[:, 0:128], warm_sb[:],
                             start=True, stop=True)

        wva = wpool.tile([128, 2, 256], BF16, tag="wva")
        wvb = wpool.tile([128, 2, 256], BF16, tag="wvb")
        wvc = wpool.tile([128, 4, 256], BF16, tag="wvc")

        def wv_lhs(c8, hp):
            t, i = (wva, c8) if c8 < 2 else (wvb, c8 - 2) if c8 < 4 \
                else (wvc, c8 - 4)
            return t[:, i, 128 * hp:128 * hp + 128]

        wo_sb = wpool.tile([128, 8, D], BF16, tag="wo")
        # block 0 is split in half-tiles so the first v-matmuls start after
        # only half of it (plus wva) has landed
        xt0a = xtp.tile([128, 4, UB], BF16, tag="xt0a")
        xt0b = xtp.tile([128, 4, UB], BF16, tag="xt0b")
        xts = [None] + [xtp.tile([128, NC8, UB], BF16, tag="xt",
                                 name=f"xt{k}") for k in range(1, NB)]

        def x_rhs(k, c8):
            if k == 0:
                t = xt0a if c8 < 4 else xt0b
                return t[:, c8 % 4, :]
            return xts[k][:, c8, :]

        # two parallel queues so the early small transfers don't serialize:
        # x stream on sync/SP, weights on the Act queue (wv first, wo after)
        nc.sync.dma_start(xt0a[:], xt_d[0, :, 0:4, :])
        nc.sync.dma_start(xt0b[:], xt_d[0, :, 4:8, :])
        for k in range(1, NB):
            nc.sync.dma_start(xts[k][:], xt_d[k])
        nc.scalar.dma_start(wva[:], wv_d[:, 0:2, :])
        nc.scalar.dma_start(wvb[:], wv_d[:, 2:4, :])
        nc.scalar.dma_start(wvc[:], wv_d[:, 4:8, :])
        for m2 in range(8):
            nc.scalar.dma_start(wo_sb[:, m2, :], wo_d[:, m2, :])

        # vt[h][64*(m%2)+di, 128*(m//2)+s] = v[t=16s+m, 256g+64h+di], bf16
        vt = [vtp.tile([128, D], BF16, tag=f"vt{h}", name=f"vt{h}")
              for h in range(NH)]

        psA = [ps_o.tile([128, 512], F32, tag="po", name=f"psA{h}")
               for h in range(NH)]

        # gpsimd cannot access PSUM, so evacuations go on DVE + Act only
        copy_engines = [nc.vector, nc.scalar]
        ce = [0]

        def ecopy(dst, src):
            eng = copy_engines[ce[0] % 2]
            ce[0] += 1
            if eng is nc.scalar:
                eng.copy(dst, src)
            else:
                eng.tensor_copy(dst, src)

        def vblock(k):
            psv = [ps_v.tile([128, UB], F32, tag="pv", name=f"pv{k}_{hp}")
                   for hp in range(2)]
            for c8 in range(NC8):
                for hp in range(2):
                    nc.tensor.matmul(psv[hp][:], wv_lhs(c8, hp),
                                     x_rhs(k, c8),
                                     start=(c8 == 0), stop=(c8 == NC8 - 1))
            return psv

        def evac(k, psv):
            # block k holds m in {2k, 2k+1}; j = m%2 = local u//128
            for hp in range(2):
                for hh in range(2):
                    for j in range(2):
                        ecopy(vt[2 * hp + hh][64 * j:64 * j + 64,
                                              128 * k:128 * k + 128],
                              psv[hp][64 * hh:64 * hh + 64,
                                      128 * j:128 * j + 128])

        def outA(k):
            for h in range(NH):
                nc.tensor.matmul(psA[h][:], vt[h][:, 128 * k:128 * k + 128],
                                 wo_sb[:, k, 0:512],
                                 start=(k == 0), stop=(k == NB - 1))

        def flushA(h):
            ob = outp.tile([128, 512], BF16, tag="ob", name=f"obA{h}")
            ecopy(ob[:], psA[h][:])
            nc.scalar.dma_start(out_d[h, :, 0:512], ob[:])

        # stream: v-proj chases x DMAs; out-proj chunk k-1 fills PE slack
        psv_prev = vblock(0)
        evac(0, psv_prev)
        for k in range(1, NB):
            psv = vblock(k)
            evac(k, psv)
            outA(k - 1)
        outA(NB - 1)

        # queue all psA evacuations first so the psB bank-reuse waits clear
        # while the first psB groups are still accumulating
        for h in range(NH):
            flushA(h)

        # second pass: out-proj columns [512,1024) + drains
        for h in range(NH):
            psB = ps_o.tile([128, 512], F32, tag="po", name=f"psB{h}")
            for m2 in range(8):
                nc.tensor.matmul(psB[:], vt[h][:, 128 * m2:128 * m2 + 128],
                                 wo_sb[:, m2, 512:1024],
                                 start=(m2 == 0), stop=(m2 == 7))
            ob = outp.tile([128, 512], BF16, tag="ob", name=f"obB{h}")
            ecopy(ob[:], psB[:])
            nc.scalar.dma_start(out_d[h, :, 512:1024], ob[:])


def _get_module():
    global _CACHED
    if _CACHED is None:
        _CACHED = _build_module()
    return _CACHED


def kernel(x, mask, Wq, Wk, Wv, Wo):
    global LAST_RESULTS
    x = np.asarray(x, dtype=np.float32)
    Wv = np.asarray(Wv, dtype=np.float32)
    Wo = np.asarray(Wo, dtype=np.float32)

    b, t, d = x.shape
    assert (b, t, d) == (B, T, D), (b, t, d)

    # x^T with tokens permuted to u = 128m + s (original t = 16s + m),
    # laid out [k, p, c8, u] to match the SBUF tiles exactly
    xts = []
    for bb in range(B):
        xT = x[bb].T                                      # [d, t]
        xTp = xT.reshape(D, 128, 16).transpose(0, 2, 1).reshape(D, T)
        xt = xTp.reshape(NC8, 128, NB, UB).transpose(2, 1, 0, 3)
        xts.append(np.ascontiguousarray(xt).astype(BF))

    # wv[p, c8, col] = Wv[128*c8 + p, col]; per-core slice of 256 cols
    wvp = Wv.reshape(NC8, 128, D).transpose(1, 0, 2)
    # wo[p, m2, n] = Wo.T[128*m2 + p, n]
    woT = np.ascontiguousarray(
        Wo.T.reshape(8, 128, D).transpose(1, 0, 2)).astype(BF)

    in_maps = []
    for c in range(NCORES):
        bb, g = c // 4, c % 4
        in_maps.append({
            "xt": xts[bb],
            "wv": np.ascontiguousarray(
                wvp[:, :, 256 * g:256 * g + 256]).astype(BF),
            "wo": woT,
        })

    nc = _get_module()
    res = run_bass_kernel_spmd(nc, in_maps, list(range(NCORES)))
    LAST_RESULTS = res

    out = np.empty((B, T, D), np.float32)
    for c in range(NCORES):
        bb, g = c // 4, c % 4
        out[bb, 512 * g:512 * g + 512, :] = \
            np.asarray(res.results[c]["out"]).astype(np.float32).reshape(512, D)
    return out


# revision 19
# speedup vs baseline: 1.2888x; 1.0236x over previous
"""nn_MultiHeadAttention_59253368815813 on 8 TRN2 NeuronCores.

The reference module is bug-faithful to its original nn.Module in two ways
that together collapse the computation:

  1. ``o = jnp.einsum('bhtl,bthd->bhtd', A, v)`` indexes ``v`` by the QUERY
     position ``t``, not the key position ``l``. ``l`` therefore only sums
     over the softmax weights, which sum to exactly 1 per row:
     ``o[b,h,t,d] == v[b,t,h,d]``. Q, K, the mask and the softmax never
     influence the output.
  2. ``o.reshape(b, T, d)`` with no transpose scrambles (head, token) so the
     reshaped activation row tj = 128*h + s is the concatenation over
     m=0..15 of v[b, 16*s+m, h, :].

So the exact computation is  out = scramble(x @ Wv) @ Wo.T,  and the
scramble makes output rows depend on one head only.

Sharding: 2 batches x 4 head-groups. Core c = (b=c//4, g=c%4) owns batch b
and heads {4g..4g+3} = Wv columns [256g, 256g+256) and output rows
[512g, 512g+512) of batch b. Each core loads only its batch's x (4.2MB in
bf16) instead of all of x, which is what made the previous version
DMA-bound (23.3MB/core at a shared ~360GB/s).

Per core, all in bf16 (PE runs bf16 at 1 cycle/row like f32r, but DMA
halves; quantization error ~2e-3 << the 2e-2 gate):
  stream x^T (tokens permuted to u = 128m + s, t = 16s + m) in 8 blocks of
  256 tokens; v-proj psum [128,256] per head-pair chases the stream; the
  reshape scramble happens in the psum->SBUF evacuation copies (spread over
  DVE/Pool/Act engines); output-projection columns [0,512) accumulate
  interleaved with the stream (chunk k uses only v tokens of block k);
  columns [512,1024) run as a second pass after the stream, overlapping the
  output DMAs.
"""

import sys
import types

import numpy as np

_TRN_REPO = "/opt/trn_rl_repo"
if _TRN_REPO not in sys.path:
    sys.path.insert(0, _TRN_REPO)


def _install_ntff_shim():
    """antenv.axon_hooks is absent in this container; provide it so
    BASS_TRACE=1 profiling works. No-op if the real module exists."""
    try:
        import antenv  # noqa: F401
    except ImportError:
        return
    if "antenv.axon_hooks" in sys.modules:
        return
    try:
        import antenv.axon_hooks  # noqa: F401
        return
    except ImportError:
        pass
    m = types.ModuleType("antenv.axon_hooks")
    m._hook = None
    m.set_axon_ntff_profile_hook = lambda h: setattr(m, "_hook", h)
    m.get_axon_ntff_profile_hook = lambda: m._hook
    sys.modules["antenv.axon_hooks"] = m
    try:
        from trn_agent_boot.trn_boot import _ntff_profile_via_ctypes

        hook = _ntff_profile_via_ctypes("/opt/axon/libaxon_pjrt.so")
        if hook is not None:
            m.set_axon_ntff_profile_hook(hook)
    except Exception:
        pass


_install_ntff_shim()

import ml_dtypes  # noqa: E402

import concourse.mybir as mybir  # noqa: E402
import concourse.tile as tile  # noqa: E402
from concourse import bacc  # noqa: E402
from concourse.bass_utils import run_bass_kernel_spmd  # noqa: E402

F32 = mybir.dt.float32
BF16 = mybir.dt.bfloat16
BF = ml_dtypes.bfloat16

B = 2
T = 2048
D = 1024
NCORES = 8
NB = 8       # 256-token (u) blocks per batch
UB = 256     # tokens per block
NC8 = 8      # contraction chunks (d = 8*128)
NH = 4       # local heads per core

_CACHED = None
LAST_RESULTS = None


def _build_module():
    nc = bacc.Bacc("TRN2", target_bir_lowering=False, debug=False,
                   num_devices=NCORES)

    xt_d = nc.dram_tensor("xt", [NB, 128, NC8, UB], BF16,
                          kind="ExternalInput").ap()
    wv_d = nc.dram_tensor("wv", [128, NC8, 256], BF16,
                          kind="ExternalInput").ap()
    wo_d = nc.dram_tensor("wo", [128, 8, D], BF16, kind="ExternalInput").ap()
    out_d = nc.dram_tensor("out", [NH, 128, D], BF16,
                           kind="ExternalOutput").ap()

    with tile.TileContext(nc) as tc:
        _emit(nc, tc, xt_d, wv_d, wo_d, out_d)
    nc.compile()
    return nc


def _emit(nc, tc, xt_d, wv_d, wo_d, out_d):
    from contextlib import ExitStack

    ctx = ExitStack()
    with ctx:
        wpool = ctx.enter_context(tc.tile_pool(name="w", bufs=1))
        xtp = ctx.enter_context(tc.tile_pool(name="xt", bufs=NB))
        vtp = ctx.enter_context(tc.tile_pool(name="vt", bufs=1))
        outp = ctx.enter_context(tc.tile_pool(name="outsb", bufs=4))
        ps_v = ctx.enter_context(tc.tile_pool(name="ps_v", bufs=4, space="PSUM"))
        ps_o = ctx.enter_context(tc.tile_pool(name="ps_o", bufs=4, space="PSUM"))

        # PE p-state warmup: the tensor engine clocks up only after ~3us of
        # continuous work, and the first real matmul cannot start until
        # ~10us (runtime preamble + first DMAs). Run throwaway matmuls on a
        # zeroed tile during that window so real matmuls run at full clock.
        warm_sb = wpool.tile([128, 256], BF16, tag="warm")
        nc.vector.memset(warm_sb[:], 0.0)
        warm_ps = ps_v.tile([128, UB], F32, tag="pv", name="warm_ps")
        for _ in range(12):
            nc.tensor.matmul(warm_ps[:], warm_sb[:, 0:128], warm_sb[:],
                             start=True, stop=True)

        wva = wpool.tile([128, 2, 256], BF16, tag="wva")
        wvb = wpool.tile([128, 2, 256], BF16, tag="wvb")
        wvc = wpool.tile([128, 4, 256], BF16, tag="wvc")

        def wv_lhs(c8, hp):
            t, i = (wva, c8) if c8 < 2 else (wvb, c8 - 2) if c8 < 4 \
                else (wvc, c8 - 4)
            return t[:, i, 128 * hp:128 * hp + 128]

        wo_sb = wpool.tile([128, 8, D], BF16, tag="wo")
        # block 0 is split in half-tiles so the first v-matmuls start after
        # only half of it (plus wva) has landed
        xt0a = xtp.tile([128, 4, UB], BF16, tag="xt0a")
        xt0b = xtp.tile([128, 4, UB], BF16, tag="xt0b")
        xts = [None] + [xtp.tile([128, NC8, UB], BF16, tag="xt",
                                 name=f"xt{k}") for k in range(1, NB)]

        def x_rhs(k, c8):
            if k == 0:
                t = xt0a if c8 < 4 else xt0b
                return t[:, c8 % 4, :]
            return xts[k][:, c8, :]

        # two parallel queues so the early small transfers don't serialize:
        # x stream on sync/SP, weights on the Act queue (wv first, wo after)
        nc.sync.dma_start(xt0a[:], xt_d[0, :, 0:4, :])
        nc.sync.dma_start(xt0b[:], xt_d[0, :, 4:8, :])
        for k in range(1, NB):
            nc.sync.dma_start(xts[k][:], xt_d[k])
        nc.scalar.dma_start(wva[:], wv_d[:, 0:2, :])
        nc.scalar.dma_start(wvb[:], wv_d[:, 2:4, :])
        nc.scalar.dma_start(wvc[:], wv_d[:, 4:8, :])
        for m2 in range(8):
            nc.scalar.dma_start(wo_sb[:, m2, :], wo_d[:, m2, :])

        # vt[h][64*(m%2)+di, 128*(m//2)+s] = v[t=16s+m, 256g+64h+di], bf16
        vt = [vtp.tile([128, D], BF16, tag=f"vt{h}", name=f"vt{h}")
              for h in range(NH)]

        psA = [ps_o.tile([128, 512], F32, tag="po", name=f"psA{h}")
               for h in range(NH)]

        # gpsimd cannot access PSUM, so evacuations go on DVE + Act only
        copy_engines = [nc.vector, nc.scalar]
        ce = [0]

        def ecopy(dst, src):
            eng = copy_engines[ce[0] % 2]
            ce[0] += 1
            if eng is nc.scalar:
                eng.copy(dst, src)
            else:
                eng.tensor_copy(dst, src)

        def vblock(k):
            psv = [ps_v.tile([128, UB], F32, tag="pv", name=f"pv{k}_{hp}")
                   for hp in range(2)]
            for c8 in range(NC8):
                for hp in range(2):
                    nc.tensor.matmul(psv[hp][:], wv_lhs(c8, hp),
                                     x_rhs(k, c8),
                                     start=(c8 == 0), stop=(c8 == NC8 - 1))
            return psv

        def evac(k, psv):
            # block k holds m in {2k, 2k+1}; j = m%2 = local u//128
            for hp in range(2):
                for hh in range(2):
                    for j in range(2):
                        ecopy(vt[2 * hp + hh][64 * j:64 * j + 64,
                                              128 * k:128 * k + 128],
                              psv[hp][64 * hh:64 * hh + 64,
                                      128 * j:128 * j + 128])

        def outA(k):
            for h in range(NH):
                nc.tensor.matmul(psA[h][:], vt[h][:, 128 * k:128 * k + 128],
                                 wo_sb[:, k, 0:512],
                                 start=(k == 0), stop=(k == NB - 1))

        def flushA(h):
            ob = outp.tile([128, 512], BF16, tag="ob", name=f"obA{h}")
            ecopy(ob[:], psA[h][:])
            nc.scalar.dma_start(out_d[h, :, 0:512], ob[:])

        # stream: v-proj chases x DMAs; out-proj chunk k-1 fills PE slack
        psv_prev = vblock(0)
        evac(0, psv_prev)
        for k in range(1, NB):
            psv = vblock(k)
            evac(k, psv)
            outA(k - 1)
        outA(NB - 1)

        # queue all psA evacuations first so the psB bank-reuse waits clear
        # while the first psB groups are still accumulating
        for h in range(NH):
            flushA(h)

        # second pass: out-proj columns [512,1024) + drains
        for h in range(NH):
            psB = ps_o.tile([128, 512], F32, tag="po", name=f"psB{h}")
            for m2 in range(8):
                nc.tensor.matmul(psB[:], vt[h][:, 128 * m2:128 * m2 + 128],
                                 wo_sb[:, m2, 512:1024],
                                 start=(m2 == 0), stop=(m2 == 7))
            ob = outp.tile([128, 512], BF16, tag="ob", name=f"obB{h}")
            ecopy(ob[:], psB[:])
            nc.scalar.dma_start(out_d[h, :, 512:1024], ob[:])


def _get_module():
    global _CACHED
    if _CACHED is None:
        _CACHED = _build_module()
    return _CACHED


def kernel(x, mask, Wq, Wk, Wv, Wo):
    global LAST_RESULTS
    x = np.asarray(x, dtype=np.float32)
    Wv = np.asarray(Wv, dtype=np.float32)
    Wo = np.asarray(Wo, dtype=np.float32)

    b, t, d = x.shape
    assert (b, t, d) == (B, T, D), (b, t, d)

    # x^T with tokens permuted to u = 128m + s (original t = 16s + m),
    # laid out [k, p, c8, u] to match the SBUF tiles exactly
    xts = []
    for bb in range(B):
        xT = x[bb].T                                      # [d, t]
        xTp = xT.reshape(D, 128, 16).transpose(0, 2, 1).reshape(D, T)
        xt = xTp.reshape(NC8, 128, NB, UB).transpose(2, 1, 0, 3)
        xts.append(np.ascontiguousarray(xt).astype(BF))

    # wv[p, c8, col] = Wv[128*c8 + p, col]; per-core slice of 256 cols
    wvp = Wv.reshape(NC8, 128, D).transpose(1, 0, 2)
    # wo[p, m2, n] = Wo.T[128*m2 + p, n]
    woT = np.ascontiguousarray(
        Wo.T.reshape(8, 128, D).transpose(1, 0, 2)).astype(BF)

    in_maps = []
    for c in range(NCORES):
        bb, g = c // 4, c % 4
        in_maps.append({
            "xt": xts[bb],
            "wv": np.ascontiguousarray(
                wvp[:, :, 256 * g:256 * g + 256]).astype(BF),
            "wo": woT,
        })

    nc = _get_module()
    res = run_bass_kernel_spmd(nc, in_maps, list(range(NCORES)))
    LAST_RESULTS = res

    out = np.empty((B, T, D), np.float32)
    for c in range(NCORES):
        bb, g = c // 4, c % 4
        out[bb, 512 * g:512 * g + 512, :] = \
            np.asarray(res.results[c]["out"]).astype(np.float32).reshape(512, D)
    return out
